# revision 37
# baseline (speedup 1.0000x reference)
# Tensor-parallel fused attention kernel for TRN2, 8 cores.
# Core r owns heads {2r, 2r+1}. Inputs per core:
#   x     [2*S, 1024] f32  (replicated; rows = b*S + s)
#   wqkv  [1024, 512] f32  (columns: q_h1|q_h2|k_h1|k_h2|v_h1|v_h2|g_h1|g_h2,
#                           64 each; q/k pre-centered per head and rope-pair
#                           permuted: [even dims 0:32 | odd dims 32:64])
#   wout  [1024, 128] f32  (w_out columns 128r:128r+128)
#   freqs [S, 32] f32      (replicated)
#   lnp   [8, 64] f32      ([qn_w, qn_w, kn_w, kn_w, qn_b, qn_b, kn_b, kn_b],
#                           rope-pair permuted; used only in affine builds)
#   sel2  [2, 128] f32     (unused; kept for input-map compat)
# Output per core:
#   out   [128, 2*S] f32   (out^T slice: rows = w_out columns owned by this core)
import math

import concourse.bass as bass
import concourse.mybir as mybir
from concourse import bacc, tile

F32 = mybir.dt.float32
F16 = mybir.dt.float16
AF = mybir.ActivationFunctionType
ALU = mybir.AluOpType
AX = mybir.AxisListType

DIM = 1024
HD = 64
EPS = 1e-5


def build(S: int, n_cores: int = 8, reps: int = 1, apply_ln_affine: bool = False):
    TB = S // 128             # token tiles per batch
    TT = 2 * TB
    JT = S // 64              # 64-wide kj blocks per batch
    QW = 512                  # q tile width
    QT = S // QW              # q tiles per batch
    GRP = 8                   # token tiles per LN/rope group
    NG = TB // GRP            # groups per batch
    NCH = 2 * QT              # allgather chunks total

    nc = bacc.Bacc("TRN2", target_bir_lowering=False, debug=False, num_devices=n_cores)

    X = nc.dram_tensor("x", [2 * S, DIM], F32, kind="ExternalInput")
    WQKV = nc.dram_tensor("wqkv", [DIM, 512], F32, kind="ExternalInput")
    WOUT = nc.dram_tensor("wout", [DIM, 128], F32, kind="ExternalInput")
    FREQS = nc.dram_tensor("freqs", [S, 32], F32, kind="ExternalInput")
    LNP = nc.dram_tensor("lnp", [8, HD], F32, kind="ExternalInput")
    SEL2 = nc.dram_tensor("sel2", [2, 128], F32, kind="ExternalInput")
    OUT = nc.dram_tensor("out", [128, 2 * S], F32, kind="ExternalOutput")

    ag_in = [nc.dram_tensor(f"ag_in{g}", [128, QW], F16) for g in range(NCH)]
    ag_out = [nc.dram_tensor(f"ag_out{g}", [8 * 128, QW], F16, addr_space="Shared")
              for g in range(NCH)]

    with tile.TileContext(nc) as tc:
        with (
            tc.tile_pool(name="persist", bufs=1) as pp,
            tc.tile_pool(name="work", bufs=2) as wp,
            tc.tile_pool(name="espool", bufs=4) as ep,
            tc.tile_pool(name="xload", bufs=6) as xp,
            tc.tile_pool(name="small", bufs=2) as sp,
            tc.tile_pool(name="ps_a", bufs=2, space="PSUM") as ps_a,
            tc.tile_pool(name="ps_s", bufs=2, space="PSUM") as ps_s,
            tc.tile_pool(name="ps_o", bufs=2, space="PSUM") as ps_o,
        ):
            # prefetch the first x tiles ahead of the weight transfers
            prefetched = {}
            for Tl in range(3):
                xt32p = xp.tile([128, DIM], F32, tag="xt32", bufs=3,
                                name="xt32p")
                if Tl % 2 == 0:
                    nc.scalar.dma_start(xt32p[:], X[Tl * 128:(Tl + 1) * 128, :])
                else:
                    nc.sync.dma_start(xt32p[:], X[Tl * 128:(Tl + 1) * 128, :])
                prefetched[(0, Tl)] = xt32p

            # ---- weights: f32 split across both HWDGE queues, cast on Act
            w16 = pp.tile([128, 8, 512], F16, tag="w16")
            for c in range(8):
                w32 = wp.tile([128, 512], F32, tag="w32", bufs=2)
                if c % 2 == 0:
                    nc.scalar.dma_start(w32[:], WQKV[c * 128:(c + 1) * 128, :])
                else:
                    nc.sync.dma_start(w32[:], WQKV[c * 128:(c + 1) * 128, :])
                nc.scalar.copy(w16[:, c, :], w32[:])
            wo32 = wp.tile([128, 8 * 128], F32, tag="wo32", bufs=1)
            nc.sync.dma_start(
                wo32[:].rearrange("p (c n) -> p c n", n=128),
                WOUT.ap().rearrange("(c p) n -> p c n", p=128))
            w16o = pp.tile([128, 8, 128], F16, tag="w16o")
            nc.scalar.copy(
                w16o[:], wo32[:].rearrange("p (c n) -> p c n", n=128))

            # cos/sin tables [128, TB*32] f16 (position s = p + 128*a)
            ftile = sp.tile([128, TB * 32], F32, tag="ftile")
            nc.gpsimd.dma_start(
                ftile[:].rearrange("p (a c) -> p a c", c=32),
                bass.AP(FREQS.ap().tensor, 0, [[32, 128], [128 * 32, TB], [1, 32]]),
            )
            sin_t = pp.tile([128, TB * 32], F16, tag="sin_t")
            cos_t = pp.tile([128, TB * 32], F16, tag="cos_t")
            halfpi = sp.tile([128, 1], F32, tag="halfpi")
            nc.vector.memset(halfpi[:], math.pi / 2)
            nc.scalar.activation(sin_t[:], ftile[:], AF.Sin)
            nc.scalar.activation(cos_t[:], ftile[:], AF.Sin, bias=halfpi[:])

            # constants
            ones_bd = pp.tile([128, 2], F16, tag="ones_bd")
            nc.vector.memset(ones_bd[:], 0.0)
            nc.vector.memset(ones_bd[0:64, 0:1], 2.0)
            nc.vector.memset(ones_bd[64:128, 1:2], 2.0)
            sel2 = pp.tile([2, 128], F16, tag="sel2")
            nc.gpsimd.dma_start(sel2[:], SEL2[:])

            if apply_ln_affine:
                lnp1 = sp.tile([1, 512], F32, tag="lnp1")
                nc.gpsimd.dma_start(
                    lnp1[:], LNP.ap().rearrange("a b -> (a b)").unsqueeze(0))
                ones1 = sp.tile([1, 128], F32, tag="ones1")
                nc.vector.memset(ones1[:], 1.0)
                with tc.tile_pool(name="pbc", bufs=1, space="PSUM") as pbc:
                    lnb_ps = pbc.tile([128, 512], F32)
                    nc.tensor.matmul(lnb_ps[:], ones1[:], lnp1[:],
                                     start=True, stop=True)
                    lnwb = pp.tile([128, 512], F16, tag="lnwb")
                    nc.vector.tensor_copy(lnwb[:], lnb_ps[:])

            # ---- persistent activations ----
            # qkgT[:, T, 0, :]=q^T, [:,T,1,:]=k^T, [:,T,2,:]=sigmoid(g)^T
            qkgT = pp.tile([128, TT, 3, 128], F16, tag="qkgT")
            og = pp.tile([128, 2 * S], F16, tag="og")
            k_bd = pp.tile([128, 2 * JT, 128], F16, tag="k_bd")
            v_bd = pp.tile([128, 2 * JT, 128], F16, tag="v_bd")
            nc.vector.memset(k_bd[:], 0.0)
            nc.vector.memset(v_bd[:], 0.0)
            # raw (pre-rope) q|k, f16, scaled in place by rstd
            xraw = pp.tile([128, TT * 256], F16, tag="xraw")
            ssq = pp.tile([128, TT * 4], F32, tag="ssq")
            rstd16 = pp.tile([128, TT * 4], F16, tag="rstd16")
            # rope staging: 2 group-parity slots x GRP tiles x (q|k|g)
            qkg16 = pp.tile([128, 2, GRP, 384], F16, tag="qkg16")

            xstep = xraw[:].ap[0][0]
            qstep = qkg16[:].ap[0][0]
            cstep = cos_t[:].ap[0][0]
            rstep = rstd16[:].ap[0][0]

            for _rep in range(reps):
                # ---------- emission helpers ----------
                def emit_qkv_T(b, Tl):
                    """One token tile: load, cast, transpose, qkv matmul, evac."""
                    T = b * TB + Tl
                    if (b, Tl) in prefetched:
                        xt32 = prefetched.pop((b, Tl))
                    else:
                        xt32 = xp.tile([128, DIM], F32, tag="xt32", bufs=3)
                        # alternate the two HWDGE queues for x transfers
                        if Tl % 2 == 0:
                            nc.scalar.dma_start(xt32[:],
                                                X[T * 128:(T + 1) * 128, :])
                        else:
                            nc.sync.dma_start(xt32[:],
                                              X[T * 128:(T + 1) * 128, :])
                    xt16 = xp.tile([128, DIM], F16, tag="xt16", bufs=3)
                    nc.vector.tensor_copy(xt16[:], xt32[:])
                    xT16 = xp.tile([128, 8, 128], F16, tag="xT16", bufs=3)
                    nc.sync.dma_start_transpose(xT16[:], xt16[:])

                    psq = ps_a.tile([128, 512], F32, tag="psa")
                    for c in range(8):
                        nc.tensor.matmul(psq[:], xT16[:, c, :], w16[:, c, :],
                                         start=(c == 0), stop=(c == 7))

                    # evacuation (PSUM readers: DVE + Act only)
                    xrT = xraw[:, T * 256:(T + 1) * 256]
                    nc.vector.tensor_copy(xrT, psq[:, 0:256])
                    sq16 = sp.tile([128, 256], F16, tag="sq16")
                    nc.gpsimd.tensor_tensor(sq16[:], xrT, xrT, ALU.mult)
                    nc.vector.tensor_reduce(
                        ssq[:, T * 4:(T + 1) * 4],
                        sq16[:].rearrange("p (a c) -> p a c", c=HD),
                        AX.X, ALU.add)
                    gp = (T // GRP) % 2
                    gdst = qkg16[:, gp, Tl % GRP, 256:384]
                    nc.scalar.activation(gdst, psq[:, 384:512], AF.Tanh,
                                         scale=0.5)
                    J0 = b * JT + 2 * (Tl % TB)
                    nc.vector.tensor_copy(v_bd[0:64, J0, 0:64],
                                          psq[0:64, 256:320])
                    nc.vector.tensor_copy(v_bd[64:128, J0, 64:128],
                                          psq[0:64, 320:384])
                    nc.vector.tensor_copy(v_bd[0:64, J0 + 1, 0:64],
                                          psq[64:128, 256:320])
                    nc.vector.tensor_copy(v_bd[64:128, J0 + 1, 64:128],
                                          psq[64:128, 320:384])

                def emit_group_tail(b, g):
                    """rstd (Newton rsqrt), scale, rope, transpose, k_bd."""
                    T0 = b * TB + g * GRP
                    gp = (T0 // GRP) % 2
                    c0 = T0 * 4
                    sv = ssq[:, c0:c0 + GRP * 4]
                    w_ = wp.tile([128, GRP * 4], F32, tag="nw")
                    y_ = wp.tile([128, GRP * 4], F32, tag="ny")
                    t_ = wp.tile([128, GRP * 4], F32, tag="nt")
                    nc.vector.tensor_scalar(w_[:], sv, 1.0 / HD, EPS,
                                            ALU.mult, ALU.add)
                    nc.vector.tensor_scalar(y_[:], w_[:], -0.5, 1.5,
                                            ALU.mult, ALU.add)
                    nc.vector.tensor_scalar_max(y_[:], y_[:], 0.2)
                    for _ in range(3):
                        nc.vector.tensor_tensor(t_[:], y_[:], y_[:], ALU.mult)
                        nc.vector.tensor_tensor(t_[:], t_[:], w_[:], ALU.mult)
                        nc.vector.tensor_scalar(t_[:], t_[:], -0.5, 1.5,
                                                ALU.mult, ALU.add)
                        nc.vector.tensor_tensor(y_[:], y_[:], t_[:], ALU.mult)
                    nc.vector.tensor_copy(rstd16[:, c0:c0 + GRP * 4], y_[:])

                    # in-place scale on Pool: xraw *= rstd (bcast over 64 dims)
                    xbase = xraw[:, T0 * 256].offset
                    xg = bass.AP(xraw.tensor, xbase,
                                 [[xstep, 128], [256, GRP], [64, 4], [1, HD]])
                    rb = bass.AP(rstd16.tensor, rstd16[:, c0].offset,
                                 [[rstep, 128], [4, GRP], [1, 4], [0, HD]])
                    nc.gpsimd.tensor_tensor(xg, xg, rb, ALU.mult)
                    if apply_ln_affine:
                        lb = bass.AP(lnwb.tensor, lnwb[:].offset,
                                     [[lnwb[:].ap[0][0], 128], [0, GRP],
                                      [1, 256]])
                        xg3 = bass.AP(xraw.tensor, xbase,
                                      [[xstep, 128], [256, GRP], [1, 256]])
                        nc.gpsimd.tensor_tensor(xg3, xg3,
                                                bass.AP(lnwb.tensor,
                                                        lnwb[:].offset,
                                                        [[lnwb[:].ap[0][0], 128],
                                                         [0, GRP], [1, 256]]),
                                                ALU.mult)
                        nc.gpsimd.tensor_tensor(
                            xg3, xg3,
                            bass.AP(lnwb.tensor, lnwb[:].offset + 256,
                                    [[lnwb[:].ap[0][0], 128], [0, GRP],
                                     [1, 256]]),
                            ALU.add)

                    # rope on DVE (all f16 packed -> 2x mode)
                    st0 = (g * GRP) * 32
                    x1 = bass.AP(xraw.tensor, xbase,
                                 [[xstep, 128], [256, GRP], [64, 4], [1, 32]])
                    x2 = bass.AP(xraw.tensor, xbase + 32,
                                 [[xstep, 128], [256, GRP], [64, 4], [1, 32]])
                    qb = qkg16[:, gp, 0, 0].offset
                    qe = bass.AP(qkg16.tensor, qb,
                                 [[qstep, 128], [384, GRP], [64, 4], [1, 32]])
                    qo = bass.AP(qkg16.tensor, qb + 32,
                                 [[qstep, 128], [384, GRP], [64, 4], [1, 32]])
                    cosb = bass.AP(cos_t.tensor, cos_t[:, st0].offset,
                                   [[cstep, 128], [32, GRP], [0, 4], [1, 32]])
                    sinb = bass.AP(sin_t.tensor, sin_t[:, st0].offset,
                                   [[cstep, 128], [32, GRP], [0, 4], [1, 32]])
                    t1 = wp.tile([128, GRP * 128], F16, tag="rt1")
                    t2 = wp.tile([128, GRP * 128], F16, tag="rt2")
                    t13 = t1[:].rearrange("p (a s c) -> p a s c", s=4, c=32)
                    t23 = t2[:].rearrange("p (a s c) -> p a s c", s=4, c=32)
                    nc.gpsimd.tensor_tensor(t13, x1, cosb, ALU.mult)
                    nc.gpsimd.tensor_tensor(t23, x2, sinb, ALU.mult)
                    nc.gpsimd.tensor_tensor(qe, t13, t23, ALU.subtract)
                    nc.gpsimd.tensor_tensor(t13, x1, sinb, ALU.mult)
                    nc.gpsimd.tensor_tensor(t23, x2, cosb, ALU.mult)
                    nc.gpsimd.tensor_tensor(qo, t13, t23, ALU.add)

                    # transposes (k_bd copies deferred to emit_kbd_unit)
                    for i in range(GRP):
                        T = T0 + i
                        nc.sync.dma_start_transpose(qkgT[:, T, :, :],
                                                    qkg16[:, gp, i, :])

                def emit_kbd_unit(b, Tl0, nT=2):
                    for Tl in range(Tl0, Tl0 + nT):
                        T = b * TB + Tl
                        J0 = b * JT + 2 * Tl
                        for jj in range(2):
                            off = 64 * jj
                            if b == 0:
                                nc.scalar.copy(k_bd[0:64, J0 + jj, 0:64],
                                               qkgT[0:64, T, 1, off:off + 64])
                                nc.vector.tensor_copy(
                                    k_bd[64:128, J0 + jj, 64:128],
                                    qkgT[64:128, T, 1, off:off + 64])
                            else:
                                nc.gpsimd.tensor_copy(
                                    k_bd[0:64, J0 + jj, 0:64],
                                    qkgT[0:64, T, 1, off:off + 64])
                                nc.gpsimd.tensor_copy(
                                    k_bd[64:128, J0 + jj, 64:128],
                                    qkgT[64:128, T, 1, off:off + 64])

                def emit_plusone(b):
                    # gate sections hold tanh(g/2); make them tanh+1 = 2*sigmoid
                    gsb = qkgT[:, b * TB:(b + 1) * TB, 2, :]
                    nc.vector.tensor_scalar_add(gsb, gsb, 1.0)

                # attention state per (b, Q)
                attn_state = {}

                def emit_po(b, Jb):
                    stt = attn_state
                    es = stt["esq"][Jb]
                    for jj in range(2):
                        J = Jb * 2 + jj
                        nc.tensor.matmul(stt["po"][:], v_bd[:, b * JT + J, :],
                                         es[:, jj * QW:(jj + 1) * QW],
                                         start=(J == 0), stop=(J == JT - 1),
                                         skip_group_check=True)

                def emit_attn_chunk(b, Q, Jb):
                    stt = attn_state
                    if Jb == 0:
                        stt["po"] = ps_o.tile([128, QW], F32, tag="po", name="po",
                                              bufs=1)
                        stt["acc"] = None
                        stt["esq"] = {}
                    qs2 = qkgT[:, b * TB + Q * 4: b * TB + (Q + 1) * 4, 0, :]
                    ps = ps_s.tile([128, 2 * QW], F32, tag="ps", bufs=2,
                                   name="ps")
                    for jj in range(2):
                        J = Jb * 2 + jj
                        nc.tensor.matmul(ps[:, jj * QW:(jj + 1) * QW],
                                         k_bd[:, b * JT + J, :], qs2,
                                         start=True, stop=True)
                    es = ep.tile([128, 2 * QW], F16, tag="es", name="es")
                    nc.scalar.activation(es[:], ps[:], AF.Exp, scale=0.125)
                    stt["esq"][Jb] = es
                    if stt["acc"] is None:
                        acc = ep.tile([128, 2 * QW], F16, tag="acc0", bufs=2,
                                      name="acc0")
                        stt["acc"] = acc
                        nc.vector.tensor_copy(acc[:], es[:])
                    else:
                        nc.vector.tensor_tensor(stt["acc"][:], stt["acc"][:],
                                                es[:], ALU.add)
                    if Jb >= 1:
                        emit_po(b, Jb - 1)

                def emit_epilogue(b, Q):
                    stt = attn_state
                    ch = b * QT + Q
                    acc = stt["acc"]
                    fd = wp.tile([128, QW], F16, tag="fd")
                    nc.vector.tensor_tensor(fd[:], acc[:, 0:QW],
                                            acc[:, QW:2 * QW], ALU.add)
                    pdpr = ps_s.tile([128, QW], F32, tag="pdpr", name="pdpr",
                                     bufs=1)
                    pd = pdpr[0:2, :]
                    nc.tensor.matmul(pd, ones_bd[:], fd[:],
                                     start=True, stop=True)
                    rdf = wp.tile([2, QW], F32, tag="rdf")
                    nc.vector.reciprocal_approx_fast(rdf[:], pd)
                    rd16 = wp.tile([2, QW], F16, tag="rd16")
                    nc.vector.tensor_copy(rd16[:], rdf[:])
                    pr = pdpr[:]
                    nc.tensor.matmul(pr, sel2[:], rd16[:], start=True, stop=True)
                    r32 = wp.tile([128, QW], F32, tag="r32")
                    nc.vector.tensor_copy(r32[:], pr)
                    on = wp.tile([128, QW], F32, tag="on")
                    nc.vector.tensor_tensor(on[:], stt["po"][:], r32[:], ALU.mult)
                    gs = qkgT[:, b * TB + Q * 4: b * TB + (Q + 1) * 4, 2, :]
                    nc.vector.tensor_tensor(
                        og[:, ch * QW:(ch + 1) * QW].rearrange(
                            "p (a c) -> p a c", c=128),
                        on[:].rearrange("p (a c) -> p a c", c=128),
                        gs, ALU.mult)
                    # allgather this chunk (trigger on DVE queue: Pool can
                    # head-of-line block behind paced work)
                    nc.sync.dma_start(ag_in[ch].ap(),
                                      og[:, ch * QW:(ch + 1) * QW])
                    nc.gpsimd.collective_compute(
                        "AllGather", ALU.bypass,
                        replica_groups=[list(range(n_cores))],
                        ins=[ag_in[ch].ap()], outs=[ag_out[ch].ap()],
                    )

                ogf_tiles = {}

                def emit_outproj_prefetch(ch):
                    ogf = ep.tile([128, 8, QW], F16, tag="ogf", bufs=2, name="ogf")
                    ogf_tiles[ch] = ogf
                    nc.scalar.dma_start(
                        ogf[:], ag_out[ch].ap().rearrange("(c p) n -> p c n",
                                                          p=128))

                def emit_outproj_mm(ch):
                    ogf = ogf_tiles.pop(ch)
                    pot = ps_a.tile([128, 512], F32, tag="psa")
                    potv = pot[:, 0:QW]
                    for c in range(8):
                        nc.tensor.matmul(potv, w16o[:, c, :], ogf[:, c, :],
                                         start=(c == 0), stop=(c == 7))
                    ot32 = wp.tile([128, QW], F32, tag="ot32")
                    nc.vector.tensor_copy(ot32[:], potv)
                    nc.scalar.dma_start(OUT[:, ch * QW:(ch + 1) * QW], ot32[:])

                # ---------- emission schedule ----------
                # phase 1 batch 0, group 0 + k_bd(g0) deferred among T8..T11
                for Tl in range(GRP):
                    emit_qkv_T(0, Tl)
                emit_group_tail(0, 0)
                for Tl in range(GRP, GRP + 4):
                    emit_qkv_T(0, Tl)
                    emit_kbd_unit(0, (Tl - GRP) * 2)

                # remaining phase-1 work runs as filler inside attention(b0):
                # batch-0 group 1 (needed by J>=16 and by epilogue(0,0)) is
                # urgent -> drained every chunk; batch-1 every other chunk
                fill0 = []
                for Tl in range(GRP + 4, TB):
                    fill0.append(lambda Tl=Tl: emit_qkv_T(0, Tl))
                fill0.append(lambda: emit_group_tail(0, 1))
                for u in range(4):
                    fill0.append(lambda u=u: emit_kbd_unit(0, GRP + 2 * u))
                fill0.append(lambda: emit_plusone(0))
                fill1 = []
                for Tl in range(TB):
                    fill1.append(lambda Tl=Tl: emit_qkv_T(1, Tl))
                    if (Tl + 1) % GRP == 0:
                        g = Tl // GRP
                        fill1.append(lambda g=g: emit_group_tail(1, g))
                        for u in range(4):
                            fill1.append(
                                lambda g=g, u=u: emit_kbd_unit(1, g * GRP + 2 * u))
                fill1.append(lambda: emit_plusone(1))

                slot = 0
                for Q in range(QT):
                    for Jb in range(JT // 2):
                        emit_attn_chunk(0, Q, Jb)
                        slot += 1
                        if fill0:
                            fill0.pop(0)()
                        elif fill1 and slot % 2 == 0:
                            fill1.pop(0)()
                    emit_po(0, JT // 2 - 1)
                    emit_epilogue(0, Q)
                while fill1:
                    fill1.pop(0)()

                # attention batch 1, outproj chunks as filler
                for Q in range(QT):
                    for Jb in range(JT // 2):
                        emit_attn_chunk(1, Q, Jb)
                        if Jb == 4 and Q < 2:
                            emit_outproj_prefetch(Q)      # b0 chunks 0,1
                        if Jb == 12 and Q < 2:
                            emit_outproj_mm(Q)
                        if Jb == 2 and Q >= 2:
                            emit_outproj_prefetch(Q)      # b0 chunks 2,3
                        if Jb == 6 and Q >= 2:
                            emit_outproj_mm(Q)
                        if Jb == 10 and Q >= 2:
                            emit_outproj_prefetch(4 + Q - 2)  # b1 chunks 4,5
                        if Jb == 14 and Q >= 2:
                            emit_outproj_mm(4 + Q - 2)
                    emit_po(1, JT // 2 - 1)
                    emit_epilogue(1, Q)
                # tail: chunks 6, 7
                for ch in (6, 7):
                    emit_outproj_prefetch(ch)
                    emit_outproj_mm(ch)

    nc.compile()
    return nc


def shard_inputs(x, freqs, w_qkv, w_out, qn_w, qn_b, kn_w, kn_b, n_cores=8):
    import numpy as np
    B, S, _ = x.shape
    x2 = np.ascontiguousarray(x.reshape(2 * S, DIM), dtype=np.float32)
    perm = np.concatenate([np.arange(0, HD, 2), np.arange(1, HD, 2)])
    lnp_base = np.stack([qn_w[perm], qn_w[perm], kn_w[perm], kn_w[perm],
                         qn_b[perm], qn_b[perm], kn_b[perm], kn_b[perm]]
                        ).astype(np.float32)
    sel2c = np.zeros((2, 128), np.float32)
    sel2c[0, 0:64] = 1.0
    sel2c[1, 64:128] = 1.0
    w_qkv = np.asarray(w_qkv, dtype=np.float32)
    maps = []
    for r in range(n_cores):
        cols = []
        for sec in range(4):
            c0 = sec * DIM + 128 * r
            blk = np.array(w_qkv[:, c0:c0 + 128])
            if sec < 2:  # q, k: center per head + rope-pair permute
                for h in range(2):
                    hb = blk[:, h * HD:(h + 1) * HD]
                    hb = hb - hb.mean(axis=1, keepdims=True)
                    blk[:, h * HD:(h + 1) * HD] = hb[:, perm]
            cols.append(blk)
        wq = np.ascontiguousarray(np.concatenate(cols, axis=1), dtype=np.float32)
        wo = np.ascontiguousarray(w_out[:, 128 * r:128 * (r + 1)], dtype=np.float32)
        maps.append({
            "x": x2, "wqkv": wq, "wout": wo,
            "freqs": np.ascontiguousarray(freqs, dtype=np.float32),
            "lnp": lnp_base, "sel2": sel2c,
        })
    return maps


def unshard_output(results, S):
    import numpy as np
    outT = np.concatenate([r["out"] for r in results], axis=0)  # [1024, 2S]
    return np.ascontiguousarray(outT.T).reshape(2, S, DIM)


_NC_CACHE = {}


def _get_nc(S, affine):
    key = (S, affine)
    if key not in _NC_CACHE:
        _NC_CACHE[key] = build(S, apply_ln_affine=affine)
    return _NC_CACHE[key]


def kernel(x, freqs, w_qkv, w_out, qn_w, qn_b, kn_w, kn_b):
    """Full-input entrypoint: shards across 8 neuron cores, runs, gathers."""
    import numpy as np
    from concourse.bass_utils import run_bass_kernel_spmd

    x = np.asarray(x, dtype=np.float32)
    freqs = np.asarray(freqs, dtype=np.float32)
    w_qkv = np.asarray(w_qkv, dtype=np.float32)
    w_out = np.asarray(w_out, dtype=np.float32)
    qn_w, qn_b = np.asarray(qn_w), np.asarray(qn_b)
    kn_w, kn_b = np.asarray(kn_w), np.asarray(kn_b)
    B, S, _ = x.shape
    affine = not (np.all(qn_w == 1) and np.all(qn_b == 0)
                  and np.all(kn_w == 1) and np.all(kn_b == 0))
    nc = _get_nc(S, bool(affine))
    maps = shard_inputs(x, freqs, w_qkv, w_out, qn_w, qn_b, kn_w, kn_b)
    res = run_bass_kernel_spmd(nc, maps, list(range(8)))
    return unshard_output(res.results, S)
